# revision 22
# baseline (speedup 1.0000x reference)
"""Trainium2 Bass kernel for nn_Attention (LayerNorm -> MHA -> out-proj).

Full (unsharded) inputs in, full output out. Shards across 8 NeuronCores as
(batch b in 0..3) x (head-group g in 0..1): core c = 2*b + g computes batch b,
heads [g*8, g*8+8), producing a partial projection [2048, 1024] (fp16); the
host sums the two group partials per batch and adds b_out.

LayerNorm runs on the host (cheap: 8M elements fp32) and the device receives
the pre-normalized, pre-transposed activations xnT [1024(dim), 2048(tok)]
fp16. The device pipeline is sized so ScalarE does nothing but the 33.5M
softmax exps (the hard floor: 1 elem/cycle/partition at 1.2 GHz ~ 293us
with [128,1024] tiles) while PE work (~270us: QKV, pair-packed S, PV with
fused denominator, out-proj) hides underneath it:

  1. Q^T/K^T [512, 2048] per head-pair (2 heads per 128-row tile) and
     V [tok, 8*(64+1)] (65th col = 1.0 -> PV also yields the softmax denom),
     all plain matmuls on xnT; DVE evacuates PSUM with a single cast-copy.
  2. Attention per head-pair p: S pair-packed via PE row-tiling (contraction
     64: head 2p in array rows 0-63 runs concurrently with head 2p+1 in rows
     64-127, ~4ns apart); one exp per (kb, qq) over both heads' scores
     [128, 1024]; PV with M=65. The PV matmuls lag the S/exp stream by
     PV_LAG steps so the PE never stalls at window boundaries waiting for
     the (DVE) PSUM evacuation of the previous window's accumulators.
  3. Normalize: one [65,512] DVE copy per head frees the PSUM bank (den row
     included), DVE reciprocal of the denom row, GPSIMD partition-broadcast,
     DVE multiply -> outT fp16.
  4. Projection outT.T @ wo per token tile, fp16 out, summed on host.

  Background work (Q/K for later pairs, projection of finished quarters) is
  drained with an explicit per-window budget, ~1 unit (2 matmuls) per kb
  step, so no window's PE load spikes above the exp cadence. A short burst
  of dummy matmuls at kernel start warms the PE HAM clock gate during the
  initial DMA wait. DMA issues ride sync/vector/gpsimd queues only -- never
  ScalarE. PSUM: sps 2x[128,1024] (4 banks) + pvA/pvB (2) + qk/v/proj ring
  (2) = 8.
"""

import sys

if "/opt/trn_rl_repo" not in sys.path:
    sys.path.insert(0, "/opt/trn_rl_repo")

import numpy as np

import concourse.tile as tile
from concourse import bacc, mybir
from concourse.bass_utils import run_bass_kernel_spmd

P = 128
N_TOK = 2048
DIM = 1024
HEADS_TOTAL = 16
H = 8  # heads per core
DH = 64
GI = H * DH  # 512, per-core inner size
INNER = HEADS_TOTAL * DH  # 1024
N_CORES = 8
SCALE = DH ** -0.5
EPS = 1e-5
NDT = DIM // P  # 8 d-tiles
NQB = N_TOK // 512  # 4 token quarters
NTT = N_TOK // P  # 16 token tiles
N_WARM = 20  # dummy matmuls to engage the PE clock gate during DMA wait

AF = mybir.ActivationFunctionType
ALU = mybir.AluOpType
f32 = mybir.dt.float32
fp16 = mybir.dt.float16

_CACHE = {}


def build_nc():
    # Calibrate the tile scheduler's DMA timing model to observed hardware
    # (first transfer lands ~9us after issue; ~290GB/s aggregate after).
    # This only shapes the static schedule -- data deps stay semaphore-
    # enforced -- but stops the scheduler from hoisting DMA-gated matmuls
    # ahead of ready ones (head-of-line PE stalls at the pipeline head).
    from concourse import hw_specs

    spec = hw_specs.TRN2Spec
    spec.DGE_DMA_DELAY = {e: d + 6000.0
                          for e, d in spec.DGE_DMA_DELAY.items()}
    spec.DMA_CYCLE = spec.DMA_CYCLE * 1.6
    spec.DMA_BUS_BYTES_PER_NS_PER_ENGINE = (
        spec.DMA_BUS_BYTES_PER_NS_PER_ENGINE / 1.6)

    nc = bacc.Bacc("TRN2", target_bir_lowering=False, debug=False)
    xt_d = nc.dram_tensor("xt", [NQB, NDT, P, 512], fp16,
                          kind="ExternalInput").ap()
    wq_d = nc.dram_tensor("wq", [4, P, NDT, P], fp16, kind="ExternalInput").ap()
    wk_d = nc.dram_tensor("wk", [4, P, NDT, P], fp16, kind="ExternalInput").ap()
    wv_d = nc.dram_tensor("wv", [NDT, P, GI], fp16, kind="ExternalInput").ap()
    wo_d = nc.dram_tensor("wo", [4, P, DIM], fp16, kind="ExternalInput").ap()
    out_d = nc.dram_tensor("out", [N_TOK, DIM], fp16, kind="ExternalOutput").ap()

    with tile.TileContext(nc) as tc:
        _body(nc, tc, xt_d, wq_d, wk_d, wv_d, wo_d, out_d)
    nc.compile()
    return nc


def _body(nc, tc, xt_d, wq_d, wk_d, wv_d, wo_d, out_d):
    # ---- persistent SBUF ----
    xT = nc.alloc_sbuf_tensor("xT", [P, NDT, N_TOK], fp16)
    QT = [nc.alloc_sbuf_tensor(f"qt{p}", [P, N_TOK], fp16) for p in range(4)]
    KT = [nc.alloc_sbuf_tensor(f"kt{p}", [P, N_TOK], fp16) for p in range(4)]
    V = nc.alloc_sbuf_tensor("vt", [P, NTT, H, DH + 1], fp16)
    outT = [nc.alloc_sbuf_tensor(f"ot{p}", [P, N_TOK], fp16) for p in range(4)]
    wq_sb = nc.alloc_sbuf_tensor("wqs", [P, NDT, GI], fp16)
    wk_sb = nc.alloc_sbuf_tensor("wks", [P, NDT, GI], fp16)
    wv_sb = nc.alloc_sbuf_tensor("wvs", [P, NDT, GI], fp16)
    wo_sb = [nc.alloc_sbuf_tensor(f"wos{p}", [P, DIM], fp16) for p in range(4)]
    ones16 = nc.alloc_sbuf_tensor("ones16", [P, 1], fp16)
    dumm = nc.alloc_sbuf_tensor("dumm", [P, 256], fp16)

    # V denominator ones-column (never overwritten by evacuations)
    nc.vector.memset(V[:, :, :, DH : DH + 1], 1.0)
    nc.vector.memset(ones16[:, :], 1.0)
    nc.vector.memset(dumm[:, :], 0.0)

    # ---- DMAs, earliest-deadline-first (the first transfer lands ~9us
    # after issue: DMA spin-up; after that the queues stream ~290GB/s
    # aggregate). ----
    dmaq3 = [nc.sync, nc.gpsimd, nc.scalar]
    dmaq = [nc.sync, nc.gpsimd]
    qi = [0]

    def dq():
        qi[0] += 1
        return dmaq3[qi[0] % 3]

    def dma_x(qb, queues=None):
        cs = slice(qb * 512, (qb + 1) * 512)
        for dt in range(NDT):
            (dq() if queues is None
             else queues[dt % len(queues)]).dma_start(
                xT[:, dt, cs], xt_d[qb, dt])

    # Strict need-time order. ScalarE gets only 3 first-wave issues (a
    # deeper scalar queue would sit in a dma_start ring-space wait and
    # block the first exp behind it). S path: x q0 + pair-0 weights
    # (1.5MB -> first exp ~17us); then x q1/q2 (KT0 tiles of window 0)
    # with wv interleaved (V tiles trail by 8 steps), x q3, cold pairs, wo.
    def dma_w(w_sb, w_d, pp, q=None):
        (q or dq()).dma_start(w_sb[:, :, pp * P : (pp + 1) * P], w_d[pp])

    for dt in range(NDT):
        dmaq[dt % 2].dma_start(xT[:, dt, 0:512], xt_d[0, dt])
    dma_w(wq_sb, wq_d, 0, nc.scalar)
    dma_w(wk_sb, wk_d, 0, nc.scalar)
    dma_x(1, dmaq)
    nc.scalar.dma_start(wv_sb[:, 0, :], wv_d[0])
    for dt in range(NDT):
        dmaq[dt % 2].dma_start(xT[:, dt, 1024:1536], xt_d[2, dt])
        if dt > 0:
            dmaq[(dt + 1) % 2].dma_start(wv_sb[:, dt, :], wv_d[dt])
    dma_x(3, dmaq)
    for pp in range(1, 4):
        dma_w(wk_sb, wk_d, pp)
    for pp in range(1, 4):
        dma_w(wq_sb, wq_d, pp)
    for p in range(4):
        dmaq[p % 2].dma_start(wo_sb[p][:, :], wo_d[p])

    # ---- pools ----
    with tc.tile_pool(name="sps", bufs=2, space="PSUM") as spool, \
         tc.tile_pool(name="pv", bufs=2, space="PSUM") as pvpool, \
         tc.tile_pool(name="qp", bufs=2, space="PSUM") as qpool, \
         tc.tile_pool(name="es", bufs=12) as espool, \
         tc.tile_pool(name="bc", bufs=2) as bcpool, \
         tc.tile_pool(name="ob", bufs=3) as obpool:

        # ---- PE warmup: engage the HAM clock gate during the DMA wait ----
        wt = qpool.tile([P, 512], f32, tag="qp", name="qp")
        for _ in range(N_WARM):
            nc.tensor.matmul(wt[0:1, 0:256], ones16[:, :], dumm[:, :],
                             start=True, stop=True)

        def qk_tile_units(dst, w_sb, pp, qb):
            """Closures (2 matmuls each + evac) for one [128,512] Q/K tile."""
            cs = slice(qb * 512, (qb + 1) * 512)
            fs = slice(pp * P, (pp + 1) * P)
            cell = {}

            def dmms(d0):
                def f():
                    if d0 == 0:
                        cell["ps"] = qpool.tile([P, 512], f32, tag="qp",
                                                name="qp")
                    for dt in (d0, d0 + 1):
                        nc.tensor.matmul(cell["ps"][:, :], w_sb[:, dt, fs],
                                         xT[:, dt, cs],
                                         start=(dt == 0), stop=(dt == 7))
                return f

            def tail():
                nc.vector.tensor_copy(dst[:, cs], cell["ps"][:, :])

            return [dmms(0), dmms(2), dmms(4), dmms(6), tail]

        def v_tile_units(t):
            """Closures for one [128 tok, 512 feat] tile of V."""
            ts = slice(t * P, (t + 1) * P)
            cell = {}

            def dmms(d0):
                def f():
                    if d0 == 0:
                        cell["ps"] = qpool.tile([P, 512], f32, tag="qp",
                                                name="qp")
                    for dt in (d0, d0 + 1):
                        nc.tensor.matmul(cell["ps"][:, :], xT[:, dt, ts],
                                         wv_sb[:, dt, :],
                                         start=(dt == 0), stop=(dt == 7))
                return f

            def tail():
                nc.vector.tensor_copy(
                    V[:, t, :, 0:DH],
                    cell["ps"][:].rearrange("p (h w) -> p h w", w=DH))

            return [dmms(0), dmms(2), dmms(4), dmms(6), tail]

        def proj_units(t):
            """Closures for output projection of token tile t; each half is
            cast and DMA'd out as soon as it completes (2 queues)."""
            ts = slice(t * P, (t + 1) * P)
            cell = {}

            def half(nn, first):
                def f():
                    if nn == 0 and first:
                        cell["ob"] = obpool.tile([P, DIM], fp16, tag="ob",
                                                 name="ob")
                    cs = slice(nn * 512, (nn + 1) * 512)
                    if first:
                        cell["pp"] = qpool.tile([P, 512], f32, tag="qp",
                                                name="qp")
                        for p in (0, 1):
                            nc.tensor.matmul(cell["pp"][:, :],
                                             outT[p][:, ts], wo_sb[p][:, cs],
                                             start=(p == 0), stop=False)
                    else:
                        for p in (2, 3):
                            nc.tensor.matmul(cell["pp"][:, :],
                                             outT[p][:, ts], wo_sb[p][:, cs],
                                             start=False, stop=(p == 3))
                        nc.vector.tensor_copy(cell["ob"][:, cs],
                                              cell["pp"][:, :])
                        q3 = dmaq3 if t >= 12 else dmaq
                        q3[(2 * t + nn) % len(q3)].dma_start(
                            out_d[ts, cs], cell["ob"][:, cs])
                return f

            return [half(0, True), half(0, False), half(1, True),
                    half(1, False)]

        # ---- attention machinery ----
        # bgq holds (deadline_window, closure): the closure MUST be emitted
        # before the first kb step of window `deadline_window` (the Tile
        # framework cannot synchronize a reader emitted before its writer).
        bgq = []
        pvq = []          # pending PV / normalize closures (staggered)
        # PV matmuls trail the S/exp stream by PVL steps: deep (10) during
        # the DMA-gated prefix so exps keep flowing while V tiles arrive,
        # shallow (4) in steady state. Must stay <= es bufs - 2.
        PVL = [10]
        npop = [0]

        def bg_pop():
            bgq.pop(0)[1]()
            npop[0] += 1

        def drain_until(w):
            while bgq and bgq[0][0] <= w:
                bg_pop()

        def kb_step(p, qq, kb, pvA, pvB):
            """S pair (row-tiled, concurrent) -> exp; queue the PV mms."""
            hA, hB = 2 * p, 2 * p + 1
            cs = slice(qq * 512, (qq + 1) * 512)
            ks = slice(kb * P, (kb + 1) * P)
            sps = spool.tile([P, 1024], f32, tag="sp", name="sp")
            nc.tensor.matmul(sps[:, 0:512], KT[p][0:DH, ks],
                             QT[p][0:DH, cs], start=True, stop=True)
            nc.tensor.matmul(sps[:, 512:1024], KT[p][DH:P, ks],
                             QT[p][DH:P, cs], start=True, stop=True)
            es = espool.tile([P, 1024], fp16, tag="es", name="es")
            nc.scalar.activation(es[:], sps[:], AF.Exp)

            def pv():
                nc.tensor.matmul(pvA[0 : DH + 1, :], V[:, kb, hA, :],
                                 es[:, 0:512],
                                 start=(kb == 0), stop=(kb == NTT - 1))
                nc.tensor.matmul(pvB[0 : DH + 1, :], V[:, kb, hB, :],
                                 es[:, 512:1024],
                                 start=(kb == 0), stop=(kb == NTT - 1))

            pvq.append(pv)
            while len(pvq) > PVL[0]:
                pvq.pop(0)()

        def normalize_closure(p, qq, pvA, pvB):
            """Evacuate pv fast (frees PSUM), then 1/denom and scale.

            For the last pair, also enqueues the projection of this token
            quarter (only emitted once outT is fully written)."""
            cs = slice(qq * 512, (qq + 1) * 512)

            def f():
                work = []
                for pv, r0 in ((pvA, 0), (pvB, DH)):
                    pvr = bcpool.tile([P, 512], f32, tag="pvr", name="pvr")
                    nc.vector.tensor_copy(pvr[0 : DH + 1, :],
                                          pv[0 : DH + 1, :])
                    work.append((pvr, r0))
                for pvr, r0 in work:
                    den = bcpool.tile([1, 512], f32, tag="den", name="den")
                    nc.vector.tensor_copy(den[:, :], pvr[DH : DH + 1, :])
                    rec = bcpool.tile([1, 512], f32, tag="rec", name="rec")
                    with nc.allow_low_precision(
                            reason="softmax denom, 18 bits"):
                        nc.vector.reciprocal_approx_fast(
                            rec[:, :], den[:, :])
                    bcs = bcpool.tile([P, 512], f32, tag="bc", name="bc")
                    nc.gpsimd.partition_broadcast(bcs[:, :], rec[0:1, :],
                                                  channels=P)
                    nc.vector.tensor_mul(outT[p][r0 : r0 + DH, cs],
                                         pvr[0:DH, :], bcs[0:DH, :])
                if p == 3:
                    for t in range(qq * 4, qq * 4 + 4):
                        bgq.extend((98, u) for u in proj_units(t))

            return f

        def pace(plan, done, step_idx):
            target = min(plan, -(-plan * (step_idx + 1) // NTT))
            while done[0] < target and bgq:
                bg_pop()
                done[0] += 1

        # ---- window order: pairs 0/1 sequential, pairs 2/3 interleaved by
        # token quarter so the projection (ready only after pair 3's
        # normalize) spreads over six windows instead of three.
        WINDOWS = ([(0, qq) for qq in range(1, NQB)]
                   + [(1, qq) for qq in range(NQB)]
                   + [w for qq in range(NQB) for w in ((2, qq), (3, qq))])

        def first_read(pred):
            return 1 + next(i for i, w in enumerate(WINDOWS) if pred(w))

        # ---- background queue: QK generation for windows 1..15 ----
        # deadline = index of the first window whose S matmuls read the tile;
        # entries must stay deadline-sorted (drain_until pops the front).
        ents = []
        for qb in range(1, NQB):
            ents.append((qb, qk_tile_units(QT[0], wq_sb, 0, qb)))
        for np1 in range(1, 4):
            kdl = first_read(lambda w, p=np1: w[0] == p)
            for qb in range(NQB):
                ents.append(
                    (kdl, qk_tile_units(KT[np1], wk_sb, np1, qb)))
            for qb in range(NQB):
                qdl = first_read(lambda w, p=np1, q=qb: w == (p, q))
                ents.append(
                    (qdl, qk_tile_units(QT[np1], wq_sb, np1, qb)))
        for dl, units in sorted(ents, key=lambda e: e[0]):
            bgq.extend((dl, u) for u in units)

        # ---- window 0 = (p0, qq0), fused with Q/K/V generation ----
        # S needs only QT[0][:, 0:512] + the KT tile of each kb (emitted
        # just before its first S); V tiles trail the S/exp stream by
        # V_LAG steps (the deep es pool lets the PV matmuls wait), so the
        # fp16 xT/wv DMAs stream in behind the fp8 S path.
        V_LAG = 8
        pvA = pvpool.tile([P, 512], f32, tag="pv", name="pv")
        pvB = pvpool.tile([P, 512], f32, tag="pv", name="pv")
        for u in qk_tile_units(QT[0], wq_sb, 0, 0):
            u()
        for kb in range(NTT):
            if kb % 4 == 0:
                for u in qk_tile_units(KT[0], wk_sb, 0, kb // 4):
                    u()
            kb_step(0, 0, kb, pvA, pvB)
            if kb >= V_LAG:
                for u in v_tile_units(kb - V_LAG):
                    u()
        pvq.append(normalize_closure(0, 0, pvA, pvB))
        vleft = [t for t in range(NTT - V_LAG, NTT)]

        # ---- windows 1..15 ----
        for wi, (p, qq) in enumerate(WINDOWS, start=1):
            if wi == 2:
                PVL[0] = 4
                while len(pvq) > PVL[0]:
                    pvq.pop(0)()
            drain_until(wi)
            due = sum(1 for dl, _ in bgq if dl <= wi + 1)
            if wi == 15:
                plan = len(bgq) + 16  # pair-3 norm adds proj mid-window
            elif wi == 1:
                plan = due  # window 1 carries the leftover V tiles
            else:
                plan = min(len(bgq) + (16 if wi >= 10 else 0), max(due, 13))
            done = [0]
            pvA = pvpool.tile([P, 512], f32, tag="pv", name="pv")
            pvB = pvpool.tile([P, 512], f32, tag="pv", name="pv")
            for kb in range(NTT):
                kb_step(p, qq, kb, pvA, pvB)
                if vleft:
                    for u in v_tile_units(vleft.pop(0)):
                        u()
                pace(plan, done, kb)
            pvq.append(normalize_closure(p, qq, pvA, pvB))
        while pvq:
            pvq.pop(0)()
        while bgq:
            bg_pop()


def _host_prep(x, ln_gamma, ln_beta, w_qkv, w_out):
    g = np.asarray(ln_gamma, dtype=np.float32)
    be = np.asarray(ln_beta, dtype=np.float32)
    W = np.asarray(w_qkv, dtype=np.float32)

    def chunks_dt(Wp):  # dt-major [8, 128, 512]: chunk dt is one 128KB block
        return np.ascontiguousarray(Wp.T.reshape(NDT, P, GI),
                                    dtype=np.float16)

    def chunks_pp(Wp):  # pair-major [4, 128, 8, 128]: 256KB per head-pair,
        # contiguous 2KB per partition line
        w = Wp.T.reshape(NDT, P, 4, P)
        return np.ascontiguousarray(w.transpose(2, 1, 0, 3),
                                    dtype=np.float16)

    in_maps = []
    for b in range(4):
        xb = np.asarray(x[b], np.float32)
        mu = xb.mean(-1, keepdims=True)
        var = xb.var(-1, keepdims=True)
        xn = (xb - mu) * (1.0 / np.sqrt(var + EPS)) * g + be
        # [qb, dt, p, c]: each (qb, dt) chunk is one contiguous 128KB block
        xtb = np.ascontiguousarray(
            xn.T.reshape(NDT, P, NQB, 512).transpose(2, 0, 1, 3),
            dtype=np.float16)

        for gr in range(2):
            lo, hi = gr * GI, (gr + 1) * GI
            Wq = W[lo:hi] * SCALE
            Wk = W[INNER + lo : INNER + hi]
            Wv = W[2 * INNER + lo : 2 * INNER + hi]
            m = {
                "xt": xtb,
                "wq": chunks_pp(Wq),
                "wk": chunks_pp(Wk),
                "wv": chunks_dt(Wv),
                "wo": np.ascontiguousarray(
                    np.asarray(w_out, np.float32)[:, lo:hi].T.reshape(
                        4, P, DIM), dtype=np.float16),
            }
            in_maps.append(m)
    return in_maps


def _run(inputs, trace=False):
    if "nc" not in _CACHE:
        _CACHE["nc"] = build_nc()
    nc = _CACHE["nc"]
    in_maps = _host_prep(inputs["x"], inputs["ln_gamma"], inputs["ln_beta"],
                         inputs["w_qkv"], inputs["w_out"])
    res = run_bass_kernel_spmd(nc, in_maps, list(range(N_CORES)), trace=trace)
    b_out = np.asarray(inputs["b_out"], dtype=np.float32)
    out = np.empty((4, N_TOK, DIM), dtype=np.float32)
    for b in range(4):
        out[b] = (res.results[2 * b]["out"].astype(np.float32)
                  + res.results[2 * b + 1]["out"].astype(np.float32)
                  + b_out[None, :])
    return out, res


def kernel(**inputs):
    out, _ = _run(inputs, trace=False)
    return out


def kernel_profiled(**inputs):
    out, res = _run(inputs, trace=True)
    return out, res
